# revision 3
# baseline (speedup 1.0000x reference)
"""GQA prefill with int8 dynamic-quant linears, distributed over 8 trn2 cores.

Sharding: DP over batch (2) x TP over head-groups (4). Core c: batch c//4,
head-group c%4 (8 q-heads, 2 kv-heads). QKV projections head-sharded; o_proj
output-column-sharded after an AllGather of the quantized attention output
(avoids the big partial-sum AllReduce; only a tiny rowmax AllReduce-max plus
the bf16 AllGather cross the cores).
"""
import numpy as np
import ml_dtypes
import concourse.bass as bass
import concourse.bacc as bacc
import concourse.mybir as mybir
import concourse.tile as tile
from concourse._compat import get_trn_type
from concourse.bass_utils import run_bass_kernel_spmd

B, S, D = 2, 1024, 4096
H, KV, HD = 32, 8, 128
TPG = 4
QHP = H // TPG        # 8 q heads / core
KVP = KV // TPG       # 2 kv heads / core
QOUT = QHP * HD       # 1024
KVOUT = KVP * HD      # 256
OC = D // TPG         # 1024 o_proj out cols / core
GROUPS = [[0, 1, 2, 3], [4, 5, 6, 7]]
SCALE = float(1.0 / np.sqrt(HD))
RC = 12582912.0       # 1.5*2^23: x+RC-RC == round-half-even(x) for |x|<2^22
NEG = -1.0e9
BF = mybir.dt.bfloat16
F32 = mybir.dt.float32
PT = 8                # pos tiles (S/128)
FT = 32               # feat tiles (D/128)

_cached = None
last_bench = None


def _build():
    nc = bacc.Bacc(get_trn_type() or "TRN2", target_bir_lowering=False)
    dp = lambda n, sh, dt: nc.declare_dram_parameter(n, sh, dt, isOutput=False)
    x = dp("x", [S, D], F32)
    wqT = dp("wqT", [D, QOUT], BF)
    wkT = dp("wkT", [D, KVOUT], BF)
    wvT = dp("wvT", [D, KVOUT], BF)
    woT = dp("woT", [D, OC], BF)
    sqv = dp("sqv", [QOUT], F32)
    bqv = dp("bqv", [QOUT], F32)
    skv = dp("skv", [KVOUT], F32)
    bkv = dp("bkv", [KVOUT], F32)
    svv = dp("svv", [KVOUT], F32)
    bvv = dp("bvv", [KVOUT], F32)
    cosT = dp("cosT", [HD, S], F32)
    sinTs = dp("sinTs", [HD, S], F32)
    diag = dp("diag", [128, 128], F32)
    ident = dp("ident", [128, 128], F32)
    onesr = dp("onesr", [1, 128], F32)
    sob = dp("sob", [128, OC], F32)
    y = nc.declare_dram_parameter("y", [S, OC], F32, isOutput=True)

    armin = nc.dram_tensor("armin", [S], F32)
    armout = nc.dram_tensor("armout", [S], F32)
    agin = nc.dram_tensor("agin", [QOUT, S], BF)
    agout = nc.dram_tensor("agout", [TPG * QOUT, S], BF)

    with tile.TileContext(nc) as tc:
        with (
            tc.tile_pool(name="const", bufs=1) as cp,
            tc.tile_pool(name="qkv", bufs=1) as qp,
        ):
            # ---- constants ----
            cosT_sb = cp.tile([HD, S], F32, tag="cosT")
            nc.sync.dma_start(cosT_sb[:], cosT[:])
            sinTs_sb = cp.tile([HD, S], F32, tag="sinTs")
            nc.sync.dma_start(sinTs_sb[:], sinTs[:])
            diag_sb = cp.tile([128, 128], F32, tag="diag")
            nc.sync.dma_start(diag_sb[:], diag[:])
            ident_sb = cp.tile([128, 128], F32, tag="ident")
            nc.sync.dma_start(ident_sb[:], ident[:])
            ones_sb = cp.tile([1, 128], F32, tag="onesr")
            nc.sync.dma_start(ones_sb[:], onesr[:])
            sob_sb = cp.tile([128, OC], F32, tag="sob")
            nc.sync.dma_start(sob_sb[:], sob[:])
            qsc, qbi = [], []
            for ot in range(QHP):
                t1 = cp.tile([128, 1], F32, tag=f"qsc{ot}")
                nc.sync.dma_start(t1[:], sqv[ot * 128:(ot + 1) * 128])
                t2 = cp.tile([128, 1], F32, tag=f"qbi{ot}")
                nc.sync.dma_start(t2[:], bqv[ot * 128:(ot + 1) * 128])
                qsc.append(t1); qbi.append(t2)
            ksc, kbi, vsc, vbi = [], [], [], []
            for ot in range(KVP):
                for (lst, src, nm) in ((ksc, skv, "ks"), (kbi, bkv, "kb"),
                                       (vsc, svv, "vs"), (vbi, bvv, "vb")):
                    t1 = cp.tile([128, 1], F32, tag=f"{nm}{ot}")
                    nc.sync.dma_start(t1[:], src[ot * 128:(ot + 1) * 128])
                    lst.append(t1)
            sxrow = cp.tile([1, S], F32, tag="sxrow")
            sxb = cp.tile([128, S], F32, tag="sxb")
            sxov = [cp.tile([128, 1], F32, tag=f"sxov{i}", name=f"sxov{i}") for i in range(PT)]

            # persistent activations
            qT = [qp.tile([128, S], BF, tag=f"qT{i}", name=f"qT{i}") for i in range(QHP)]
            kT = [qp.tile([128, S], BF, tag=f"kT{i}", name=f"kT{i}") for i in range(KVP)]
            vT = [qp.tile([128, S], BF, tag=f"vT{i}", name=f"vT{i}") for i in range(KVP)]
            vsb = [qp.tile([128, 129], BF, tag=f"vsb{i}", name=f"vsb{i}") for i in range(KVP * PT)]

            with (
                tc.tile_pool(name="xiTp", bufs=1) as xp,
                tc.tile_pool(name="qtmp", bufs=2) as tp,
                tc.tile_pool(name="ps12", bufs=4, space="PSUM") as ps1,
            ):
                xiT = [xp.tile([128, S], BF, tag=f"xiT{i}", name=f"xiT{i}") for i in range(FT)]
                # ---- phase 1: quantize x, build xiT + sx ----
                for pt in range(PT):
                    xt = tp.tile([128, D], F32, tag="xt")
                    nc.sync.dma_start(xt[:], x[pt * 128:(pt + 1) * 128, :])
                    rmax = tp.tile([128, 1], F32, tag="rmax")
                    nc.vector.reduce_max(rmax[:], xt[:], axis=mybir.AxisListType.X,
                                         apply_absolute_value=True)
                    rr = tp.tile([128, 1], F32, tag="rr")
                    nc.vector.reciprocal(rr[:], rmax[:])
                    rq = tp.tile([128, 1], F32, tag="rq")
                    nc.vector.tensor_scalar_mul(rq[:], rr[:], 127.0)
                    nc.vector.tensor_scalar(xt[:], xt[:], rq[:], None,
                                            op0=mybir.AluOpType.mult)
                    nc.vector.tensor_scalar(xt[:], xt[:], RC, -RC,
                                            op0=mybir.AluOpType.add,
                                            op1=mybir.AluOpType.add)
                    xqb = tp.tile([128, D], BF, tag="xqb")
                    nc.vector.tensor_copy(xqb[:], xt[:])
                    for ft in range(FT):
                        nc.sync.dma_start(
                            xiT[ft][:, pt * 128:(pt + 1) * 128],
                            xqb[:, ft * 128:(ft + 1) * 128], transpose=True)
                    sxc = tp.tile([128, 1], F32, tag="sxc")
                    nc.vector.tensor_scalar_mul(sxc[:], rmax[:], 1.0 / 127.0)
                    pst = ps1.tile([1, 128], F32, tag="tr", bufs=2)
                    nc.tensor.transpose(pst[:], sxc[:], ident_sb[:])
                    nc.scalar.copy(sxrow[0:1, pt * 128:(pt + 1) * 128], pst[:])
                for c in range(2):
                    psb = ps1.tile([128, 512], F32, tag="bc", bufs=2)
                    nc.tensor.matmul(psb[:], ones_sb[:],
                                     sxrow[0:1, c * 512:(c + 1) * 512],
                                     start=True, stop=True)
                    nc.scalar.copy(sxb[:, c * 512:(c + 1) * 512], psb[:])

                # ---- phase 2: QKV projections ----
                specs = [(wqT, QHP, qsc, qbi, qT), (wkT, KVP, ksc, kbi, kT),
                         (wvT, KVP, vsc, vbi, vT)]
                for (wt, nop, svec, bvec, dst) in specs:
                    for otp in range(nop // 2):
                        psA = [ps1.tile([128, 512], F32, tag="mm", bufs=4, name="psA")
                               for _ in range(4)]
                        for ft in range(FT):
                            wtl = tp.tile([128, 256], BF, tag="wtl", bufs=3)
                            nc.sync.dma_start(
                                wtl[:], wt[ft * 128:(ft + 1) * 128,
                                           otp * 256:(otp + 1) * 256])
                            for o2 in range(2):
                                for pc in range(2):
                                    nc.tensor.matmul(
                                        psA[o2 * 2 + pc][:],
                                        wtl[:, o2 * 128:(o2 + 1) * 128],
                                        xiT[ft][:, pc * 512:(pc + 1) * 512],
                                        start=(ft == 0), stop=(ft == FT - 1))
                        for o2 in range(2):
                            ot = otp * 2 + o2
                            for pc in range(2):
                                tmp = tp.tile([128, 512], F32, tag="fin", bufs=3)
                                nc.vector.tensor_mul(tmp[:], psA[o2 * 2 + pc][:],
                                                     sxb[:, pc * 512:(pc + 1) * 512])
                                nc.scalar.activation(
                                    dst[ot][:, pc * 512:(pc + 1) * 512], tmp[:],
                                    mybir.ActivationFunctionType.Identity,
                                    bias=bvec[ot][:], scale=svec[ot][:])

            # ---- phase 3: RoPE on q,k; transpose v ----
            with tc.tile_pool(name="rp", bufs=2) as rp:
                for t in qT + kT:
                    sh = rp.tile([128, S], BF, tag="sh")
                    nc.vector.tensor_copy(sh[0:64, :], t[64:128, :])
                    nc.vector.tensor_copy(sh[64:128, :], t[0:64, :])
                    ta = rp.tile([128, S], F32, tag="ta")
                    nc.vector.tensor_mul(ta[:], t[:], cosT_sb[:])
                    tb = rp.tile([128, S], F32, tag="tb")
                    nc.vector.tensor_mul(tb[:], sh[:], sinTs_sb[:])
                    nc.vector.tensor_add(t[:], ta[:], tb[:])
                for kv in range(KVP):
                    for pt in range(PT):
                        vo = vsb[kv * PT + pt]
                        nc.sync.dma_start(vo[:, 0:128],
                                          vT[kv][:, pt * 128:(pt + 1) * 128],
                                          transpose=True)
                        nc.vector.memset(vo[:, 128:129], 1.0)

            # ---- phase 4: attention ----
            aop_cm = tc.tile_pool(name="ao", bufs=1)
            aop = aop_cm.__enter__()
            ao = [aop.tile([128, QOUT], F32, tag=f"ao{i}", name=f"ao{i}")
                  for i in range(PT)]
            with (
                tc.tile_pool(name="at", bufs=2) as at,
                tc.tile_pool(name="psS", bufs=2, space="PSUM") as psS,
                tc.tile_pool(name="psO", bufs=2, space="PSUM") as psO,
            ):
                for h in range(QHP):
                    kv = h // (QHP // KVP)
                    for qt in range(PT):
                        nk = (qt + 1) * 128
                        pss = psS.tile([128, S], F32, tag="pss")
                        for kc in range((nk + 511) // 512):
                            w = min(512, nk - kc * 512)
                            nc.tensor.matmul(
                                pss[:, kc * 512:kc * 512 + w],
                                qT[h][:, qt * 128:(qt + 1) * 128],
                                kT[kv][:, kc * 512:kc * 512 + w],
                                start=True, stop=True)
                        nc.vector.tensor_add(pss[:, qt * 128:nk],
                                             pss[:, qt * 128:nk], diag_sb[:])
                        m = at.tile([128, 1], F32, tag="m")
                        nc.vector.reduce_max(m[:], pss[:, 0:nk],
                                             axis=mybir.AxisListType.X)
                        nm = at.tile([128, 1], F32, tag="nm")
                        nc.vector.tensor_scalar_mul(nm[:], m[:], -SCALE)
                        P = at.tile([128, S], BF, tag="P")
                        nc.scalar.activation(P[:, 0:nk], pss[:, 0:nk],
                                             mybir.ActivationFunctionType.Exp,
                                             bias=nm[:], scale=SCALE)
                        pso = psO.tile([128, 129], F32, tag="pso")
                        for j in range(qt + 1):
                            ptt = at.tile([128, 128], BF, tag="ptt", bufs=4)
                            nc.sync.dma_start(ptt[:], P[:, j * 128:(j + 1) * 128],
                                              transpose=True)
                            nc.tensor.matmul(pso[:], ptt[:], vsb[kv * PT + j][:],
                                             start=(j == 0), stop=(j == qt))
                        rd = at.tile([128, 1], F32, tag="rd")
                        nc.vector.reciprocal(rd[:], pso[:, 128:129])
                        nc.scalar.activation(ao[qt][:, h * 128:(h + 1) * 128],
                                             pso[:, 0:128],
                                             mybir.ActivationFunctionType.Copy,
                                             scale=rd[:])

            # ---- phase 5/6: rowmax AR, quantize attn-out, transpose, AG ----
            with tc.tile_pool(name="oq", bufs=2) as oq:
                for qt in range(PT):
                    am = oq.tile([128, 1], F32, tag="am")
                    nc.vector.reduce_max(am[:], ao[qt][:],
                                         axis=mybir.AxisListType.X,
                                         apply_absolute_value=True)
                    nc.sync.dma_start(armin[qt * 128:(qt + 1) * 128], am[:])
                nc.gpsimd.collective_compute(
                    "AllReduce", mybir.AluOpType.max, replica_groups=GROUPS,
                    ins=[armin[:]], outs=[armout[:]])
                for qt in range(PT):
                    sxo = oq.tile([128, 1], F32, tag="sxo")
                    nc.sync.dma_start(sxo[:], armout[qt * 128:(qt + 1) * 128])
                    nc.vector.tensor_scalar_mul(sxov[qt][:], sxo[:], 1.0 / 127.0)
                    rro = oq.tile([128, 1], F32, tag="rro")
                    nc.vector.reciprocal(rro[:], sxo[:])
                    rqo = oq.tile([128, 1], F32, tag="rqo")
                    nc.vector.tensor_scalar_mul(rqo[:], rro[:], 127.0)
                    tq = oq.tile([128, QOUT], F32, tag="tq")
                    nc.vector.tensor_scalar(tq[:], ao[qt][:], rqo[:], None,
                                            op0=mybir.AluOpType.mult)
                    nc.vector.tensor_scalar(tq[:], tq[:], RC, -RC,
                                            op0=mybir.AluOpType.add,
                                            op1=mybir.AluOpType.add)
                    tqb = oq.tile([128, QOUT], BF, tag="tqb")
                    nc.vector.tensor_copy(tqb[:], tq[:])
                    for fl in range(QOUT // 128):
                        xoT = oq.tile([128, 128], BF, tag="xoT", bufs=4)
                        nc.sync.dma_start(xoT[:], tqb[:, fl * 128:(fl + 1) * 128],
                                          transpose=True)
                        nc.sync.dma_start(
                            agin[fl * 128:(fl + 1) * 128,
                                 qt * 128:(qt + 1) * 128], xoT[:])
                nc.gpsimd.collective_compute(
                    "AllGather", mybir.AluOpType.bypass, replica_groups=GROUPS,
                    ins=[agin[:]], outs=[agout[:]])

            aop_cm.__exit__(None, None, None)
            # ---- phase 7: o_proj ----
            with (
                tc.tile_pool(name="wo", bufs=1) as wop,
                tc.tile_pool(name="op", bufs=3) as op,
                tc.tile_pool(name="psY", bufs=4, space="PSUM") as psY,
            ):
                woT_sb = [wop.tile([128, OC], BF, tag=f"woT{i}", name=f"woT{i}")
                          for i in range(FT)]
                for ft in range(FT):
                    nc.sync.dma_start(woT_sb[ft][:],
                                      woT[ft * 128:(ft + 1) * 128, :])
                for pt in range(PT):
                    psy = [psY.tile([128, 512], F32, tag="psy", name="psy") for _ in range(2)]
                    for ft in range(FT):
                        xo = op.tile([128, 128], BF, tag="xo")
                        nc.sync.dma_start(
                            xo[:], agout[ft * 128:(ft + 1) * 128,
                                         pt * 128:(pt + 1) * 128])
                        for occ in range(2):
                            nc.tensor.matmul(
                                psy[occ][:], xo[:],
                                woT_sb[ft][:, occ * 512:(occ + 1) * 512],
                                start=(ft == 0), stop=(ft == FT - 1))
                    for occ in range(2):
                        ty = op.tile([128, 512], F32, tag="ty")
                        nc.scalar.activation(ty[:], psy[occ][:],
                                             mybir.ActivationFunctionType.Copy,
                                             scale=sxov[pt][:])
                        yo = op.tile([128, 512], F32, tag="yo")
                        nc.vector.tensor_mul(yo[:], ty[:],
                                             sob_sb[:, occ * 512:(occ + 1) * 512])
                        nc.sync.dma_start(
                            y[pt * 128:(pt + 1) * 128,
                              occ * 512:(occ + 1) * 512], yo[:])
    nc.compile()
    return nc


def kernel(x, cos, sin, wq, sq, bq, wk, sk, bk, wv, sv, bv, wo, so):
    global _cached, last_bench
    if _cached is None:
        _cached = _build()
    nc = _cached
    bf = ml_dtypes.bfloat16
    x = np.asarray(x, np.float32)
    cosT = np.ascontiguousarray(np.asarray(cos, np.float32).T)
    sinT = np.ascontiguousarray(np.asarray(sin, np.float32).T).copy()
    sinT[:HD // 2] *= -1.0
    wq8 = np.asarray(wq).astype(np.int8); wk8 = np.asarray(wk).astype(np.int8)
    wv8 = np.asarray(wv).astype(np.int8); wo8 = np.asarray(wo).astype(np.int8)
    r, c = np.arange(128)[:, None], np.arange(128)[None, :]
    diag = np.where(c <= r, 0.0, NEG).astype(np.float32)
    ident = np.eye(128, dtype=np.float32)
    onesr = np.ones((1, 128), np.float32)
    in_maps = []
    for core in range(8):
        b, hg = core // TPG, core % TPG
        qs = slice(hg * QOUT, (hg + 1) * QOUT)
        ks = slice(hg * KVOUT, (hg + 1) * KVOUT)
        in_maps.append({
            "x": x[b],
            "wqT": np.ascontiguousarray(wq8[qs].T).astype(bf),
            "wkT": np.ascontiguousarray(wk8[ks].T).astype(bf),
            "wvT": np.ascontiguousarray(wv8[ks].T).astype(bf),
            "woT": np.ascontiguousarray(wo8[qs].T).astype(bf),
            "sqv": np.asarray(sq, np.float32)[qs],
            "bqv": np.asarray(bq, np.float32)[qs],
            "skv": np.asarray(sk, np.float32)[ks],
            "bkv": np.asarray(bk, np.float32)[ks],
            "svv": np.asarray(sv, np.float32)[ks],
            "bvv": np.asarray(bv, np.float32)[ks],
            "cosT": cosT, "sinTs": sinT, "diag": diag, "ident": ident,
            "onesr": onesr,
            "sob": np.broadcast_to(np.asarray(so, np.float32)[qs],
                                   (128, OC)).copy(),
        })
    last_bench = run_bass_kernel_spmd(nc, in_maps, list(range(8)))
    out = np.empty((B, S, D), np.float32)
    for core in range(8):
        b, hg = core // TPG, core % TPG
        out[b][:, hg * OC:(hg + 1) * OC] = last_bench.results[core]["y"]
    return out
